# revision 1
# baseline (speedup 1.0000x reference)
"""Trainium2 Bass kernel for Luong dot attention + softmax.

  queries: [1, 64, 1024] f32
  keys:    [4096, 64, 1024] f32
  out:     [1, 64, 4096] f32 = softmax_s(einsum('bh,sbh->bs', q[0], keys))

Sharding: data-parallel over batch. Core m handles batches [8m, 8m+8).
Per core this is memory-bound: 128 MiB of keys streamed from HBM.

Per-core kernel:
  - keys shard viewed as [4096, 8*1024]; streamed as 64 half-tile DMAs of
    [128, 4096] (2 MiB each, 16 KiB contiguous per partition -> near-peak
    HBM bandwidth), 6-deep buffered.
  - q replicated host-side to [128, 8192] so the DVE can read it per-lane.
  - Per half-tile: one wide DVE tensor_mul (4 batches at once), then 4
    ScalarE in-place Copy ops whose accum_out produces the [128,1] score
    columns (the fused tensor_tensor_reduce crashes this runtime's NRT).
  - Softmax over s=4096 (partitions x 32 columns): free-dim reduce_max,
    gpsimd partition_all_reduce for the cross-partition max/sum, ScalarE
    Exp with per-partition bias and fused sum-accum, reciprocal + scale.
  - Scores live as S[p, b, t] with s = t*128 + p; host untransposes.
"""

import numpy as np

N_CORES = 8
SEQ = 4096
B_TOT = 64
H = 1024
P = 128
B = B_TOT // N_CORES          # 8 batches per core
NT = SEQ // P                 # 32 s-tiles
FB = B * H                    # 8192 free elems per s-tile
CH = 4                        # batches per DVE multiply instruction

_PROGRAM = None


def _build_program():
    """Build + compile the Bass/Tile program once per process."""
    import concourse.bass as bass  # noqa: F401
    import concourse.mybir as mybir
    import concourse.bass_isa as bass_isa
    from concourse import bacc, tile

    f32 = mybir.dt.float32
    nc = bacc.Bacc("TRN2", target_bir_lowering=False, debug=False,
                   num_devices=N_CORES)

    keys_d = nc.dram_tensor("keys", [SEQ, FB], f32, kind="ExternalInput")
    qrep_d = nc.dram_tensor("qrep", [P, FB], f32, kind="ExternalInput")
    out_d = nc.dram_tensor("out", [P, B, NT], f32, kind="ExternalOutput")

    with tile.TileContext(nc) as tc:
        with (
            tc.tile_pool(name="kpool", bufs=6) as kpool,
            tc.tile_pool(name="qpool", bufs=1) as qpool,
            tc.tile_pool(name="ppool", bufs=3) as ppool,
            tc.tile_pool(name="spool", bufs=1) as spool,
        ):
            qrep = qpool.tile([P, FB], f32)
            nc.sync.dma_start(qrep[:, :], qrep_d.ap()[:, :])

            s_all = spool.tile([P, B, NT], f32)   # raw scores
            e_all = spool.tile([P, B, NT], f32)   # exp(scores - max)
            o_all = spool.tile([P, B, NT], f32)   # softmax output
            m1 = spool.tile([P, B], f32)          # per-partition max
            gm = spool.tile([P, B], f32)          # global max (all parts)
            negm = spool.tile([P, B], f32)        # -global max
            r1 = spool.tile([P, B], f32)          # per-partition exp sum
            gr = spool.tile([P, B], f32)          # global exp sum
            rr = spool.tile([P, B], f32)          # 1 / global sum

            # tensor_tensor_reduce would fuse multiply+reduce in one DVE op,
            # but it crashes the NRT exec unit on this runtime; split across
            # DVE (one wide multiply per CH batches) + ScalarE (in-place Copy
            # with row-sum accum per batch) so the reduce costs no DVE time.
            # 2 MiB half-tile DMAs (CH batches each) pipeline more finely
            # than one 4 MiB DMA per s-tile.
            HF = CH * H
            for t in range(NT):
                for h0 in range(B // CH):
                    ktile = kpool.tile([P, HF], f32, tag="ktile")
                    nc.sync.dma_start(
                        ktile[:, :],
                        keys_d.ap()[t * P:(t + 1) * P, h0 * HF:(h0 + 1) * HF])
                    pscr = ppool.tile([P, HF], f32, tag="pscr")
                    nc.vector.tensor_mul(pscr[:, :], ktile[:, :],
                                         qrep[:, h0 * HF:(h0 + 1) * HF])
                    for j in range(CH):
                        nc.scalar.activation(
                            pscr[:, j * H:(j + 1) * H],
                            pscr[:, j * H:(j + 1) * H],
                            mybir.ActivationFunctionType.Copy,
                            accum_out=s_all[:, h0 * CH + j, t:t + 1],
                        )

            # ---- softmax over s (partitions x NT columns), per batch ----
            nc.vector.reduce_max(m1[:, :], s_all[:, :, :],
                                 axis=mybir.AxisListType.X)
            nc.gpsimd.partition_all_reduce(gm[:, :], m1[:, :], channels=P,
                                           reduce_op=bass_isa.ReduceOp.max)
            nc.vector.tensor_scalar_mul(negm[:, :], gm[:, :], -1.0)
            for b in range(B):
                nc.scalar.activation(
                    e_all[:, b, :], s_all[:, b, :],
                    mybir.ActivationFunctionType.Exp,
                    bias=negm[:, b:b + 1], scale=1.0,
                    accum_out=r1[:, b:b + 1],
                )
            nc.gpsimd.partition_all_reduce(gr[:, :], r1[:, :], channels=P,
                                           reduce_op=bass_isa.ReduceOp.add)
            nc.vector.reciprocal(rr[:, :], gr[:, :])
            for b in range(B):
                nc.vector.tensor_scalar_mul(o_all[:, b, :], e_all[:, b, :],
                                            rr[:, b:b + 1])
            nc.sync.dma_start(out_d.ap()[:, :, :], o_all[:, :, :])

    nc.compile()
    return nc


def _get_program():
    global _PROGRAM
    if _PROGRAM is None:
        _PROGRAM = _build_program()
    return _PROGRAM


def _make_in_maps(queries, keys):
    queries = np.ascontiguousarray(queries, dtype=np.float32)
    keys = np.ascontiguousarray(keys, dtype=np.float32)
    in_maps = []
    for m in range(N_CORES):
        lo, hi = m * B, (m + 1) * B
        ks = np.ascontiguousarray(keys[:, lo:hi, :]).reshape(SEQ, FB)
        q = queries[0, lo:hi, :].reshape(FB)
        qrep = np.ascontiguousarray(np.broadcast_to(q, (P, FB)))
        in_maps.append({"keys": ks, "qrep": qrep})
    return in_maps


def _run(queries, keys, **spmd_kwargs):
    from concourse import bass_utils

    nc = _get_program()
    in_maps = _make_in_maps(queries, keys)
    res = bass_utils.run_bass_kernel_spmd(
        nc, in_maps, core_ids=list(range(N_CORES)), **spmd_kwargs
    )
    outs = []
    for m in range(N_CORES):
        o = np.asarray(res.results[m]["out"]).reshape(P, B, NT)
        # o[p, b, t] = score(batch m*B+b, s = t*128 + p)
        outs.append(o.transpose(1, 2, 0).reshape(B, SEQ))
    full = np.concatenate(outs, axis=0)[None]  # [1, 64, 4096]
    return np.ascontiguousarray(full.astype(np.float32)), res


def kernel(queries, keys):
    out, _ = _run(queries, keys)
    return out



# revision 2
# speedup vs baseline: 2.0956x; 2.0956x over previous
"""Trainium2 Bass kernel for Luong dot attention + softmax.

  queries: [1, 64, 1024] f32
  keys:    [4096, 64, 1024] f32
  out:     [1, 64, 4096] f32 = softmax_s(einsum('bh,sbh->bs', q[0], keys))

Sharding: data-parallel over batch. Core m handles batches [8m, 8m+8).
Per core this is memory-bound: 128 MiB of keys streamed from HBM
(measured pure-stream ceiling on these cores: ~385 GB/s with contiguous
2 MiB DMAs queued 10 deep on the sync HWDGE ring).

Per-core kernel (v4; v1 measured ~403 us steady-state, v4 ~355 us):
  - keys relaid out host-side to [2, 32, 128, 4096]: each DMA is one
    fully contiguous 2 MiB HBM block, consumed in stream order, all on
    the sync(SP) ring, 10-deep tile pool (deeper queue = the single
    biggest win over v1: 6-deep strided was ~30 us slower).
  - q loaded as [1, 8192] (32 KiB, not a 4 MiB host-replicated blob)
    and broadcast across the 128 partitions on-chip by TensorE
    ones-matmuls through PSUM while the first key tiles are in flight.
  - DVE multiplies kt *= qrep IN PLACE (no product scratch; the freed
    SBUF pays for the deeper DMA queue); ScalarE Copy-with-accum then
    reduces each batch's 1024-wide slice into the score column.
  - batches processed in 2 groups of 4: group 0's softmax (DVE
    reduce_max -> gpsimd cross-partition max -> ScalarE Exp with bias,
    fused sum accum -> TensorE ones-matmul partition sum -> reciprocal
    -> scale) overlaps group 1's key streaming, so only group 1's
    ~10 us softmax tail is exposed.
  - scores live as S[g][p, j, t] with s = t*128 + p; host untransposes.
"""

import numpy as np

N_CORES = 8
SEQ = 4096
B_TOT = 64
H = 1024
P = 128
B = B_TOT // N_CORES          # 8 batches per core
NG = 2                        # batch groups per core
GB = B // NG                  # 4 batches per group
NT = SEQ // P                 # 32 s-tiles
GF = GB * H                   # 4096 free elems per group-tile
FB = B * H                    # 8192
KBUFS = 10

_PROGRAM = None


def _build_program(reps=1, softmax_mode="gmax"):
    import concourse.mybir as mybir
    import concourse.bass_isa as bass_isa
    from concourse import bacc, tile

    f32 = mybir.dt.float32
    nc = bacc.Bacc("TRN2", target_bir_lowering=False, debug=False,
                   num_devices=N_CORES)

    keys_d = nc.dram_tensor("keys", [NG, NT, P, GF], f32,
                            kind="ExternalInput")
    q_d = nc.dram_tensor("q", [1, FB], f32, kind="ExternalInput")
    out_d = nc.dram_tensor("out", [NG, P, GB, NT], f32,
                           kind="ExternalOutput")

    QCHUNK = 512                  # one PSUM bank of f32, max moving free

    with tile.TileContext(nc) as tc:
        with (
            tc.tile_pool(name="qpool", bufs=1) as qpool,
            tc.psum_pool(name="pspool", bufs=2) as pspool,
        ):
            ones = qpool.tile([P, P], f32)
            nc.vector.memset(ones[:, :], 1.0)
            qrep = qpool.tile([P, FB], f32)

            # broadcast q across all 128 partitions: 16 ones-matmuls of
            # [1,128]x[1,512] -> PSUM [128,512] -> copy to qrep chunk.
            # q1/ones1 live in a scratch pool released before kpool opens.
            with tc.tile_pool(name="qinit", bufs=1) as qinit:
                q1 = qinit.tile([1, FB], f32)
                nc.scalar.dma_start(q1[:, :], q_d.ap()[:, :])
                ones1 = qinit.tile([1, P], f32)
                nc.vector.memset(ones1[:, :], 1.0)
                for c in range(FB // QCHUNK):
                    qp = pspool.tile([P, QCHUNK], f32, tag="qb")
                    nc.tensor.matmul(qp[:, :], ones1[:, :],
                                     q1[:, c * QCHUNK:(c + 1) * QCHUNK])
                    nc.vector.tensor_copy(
                        qrep[:, c * QCHUNK:(c + 1) * QCHUNK], qp[:, :])

            with (
                tc.tile_pool(name="kpool", bufs=KBUFS) as kpool,
                tc.tile_pool(name="spool",
                             bufs=2 if reps > 1 else 1) as spool,
            ):
                for rep in range(reps):
                    for g in range(NG):
                        s_g = spool.tile([P, GB, NT], f32, tag=f"s{g}")
                        e_g = spool.tile([P, GB, NT], f32, tag=f"e{g}")
                        o_g = spool.tile([P, GB, NT], f32, tag=f"o{g}")
                        m1 = spool.tile([P, GB], f32, tag=f"m1{g}")
                        gm = spool.tile([P, GB], f32, tag=f"gm{g}")
                        negm = spool.tile([P, GB], f32, tag=f"negm{g}")
                        r1 = spool.tile([P, GB], f32, tag=f"r1{g}")
                        rr = spool.tile([P, GB], f32, tag=f"rr{g}")

                        for t in range(NT):
                            kt = kpool.tile([P, GF], f32, tag="kt")
                            nc.sync.dma_start(kt[:, :],
                                              keys_d.ap()[g, t, :, :])
                            nc.vector.tensor_mul(
                                kt[:, :], kt[:, :],
                                qrep[:, g * GF:(g + 1) * GF])
                            for j in range(GB):
                                nc.scalar.activation(
                                    kt[:, j * H:(j + 1) * H],
                                    kt[:, j * H:(j + 1) * H],
                                    mybir.ActivationFunctionType.Copy,
                                    accum_out=s_g[:, j, t:t + 1],
                                )

                        # softmax over s for this group; overlaps the next
                        # group's key streaming.
                        nc.vector.reduce_max(m1[:, :], s_g[:, :, :],
                                             axis=mybir.AxisListType.X)
                        nc.gpsimd.partition_all_reduce(
                            gm[:, :], m1[:, :], channels=P,
                            reduce_op=bass_isa.ReduceOp.max)
                        nc.vector.tensor_scalar_mul(negm[:, :], gm[:, :],
                                                    -1.0)
                        for j in range(GB):
                            nc.scalar.activation(
                                e_g[:, j, :], s_g[:, j, :],
                                mybir.ActivationFunctionType.Exp,
                                bias=negm[:, j:j + 1], scale=1.0,
                                accum_out=r1[:, j:j + 1],
                            )
                        gr = pspool.tile([P, GB], f32, tag=f"gr{g}")
                        nc.tensor.matmul(gr[:, :], ones[:, :], r1[:, :])
                        nc.vector.reciprocal(rr[:, :], gr[:, :])
                        for j in range(GB):
                            nc.vector.tensor_scalar_mul(
                                o_g[:, j, :], e_g[:, j, :], rr[:, j:j + 1])
                        nc.scalar.dma_start(out_d.ap()[g, :, :, :],
                                            o_g[:, :, :])

    nc.compile()
    return nc


def _get_program():
    global _PROGRAM
    if _PROGRAM is None:
        _PROGRAM = _build_program()
    return _PROGRAM


def _make_in_maps(queries, keys):
    queries = np.ascontiguousarray(queries, dtype=np.float32)
    keys = np.ascontiguousarray(keys, dtype=np.float32)
    in_maps = []
    for m in range(N_CORES):
        lo = m * B
        # kstream[g, t, p, j*H + h] = keys[t*128 + p, lo + g*GB + j, h]
        ks = keys[:, lo:lo + B, :].reshape(NT, P, NG, GB, H)
        ks = np.ascontiguousarray(ks.transpose(2, 0, 1, 3, 4)).reshape(
            NG, NT, P, GF)
        q = np.ascontiguousarray(
            queries[0, lo:lo + B, :].reshape(1, FB))
        in_maps.append({"keys": ks, "q": q})
    return in_maps


def _unpack_out(res_out_list):
    outs = []
    for m in range(N_CORES):
        o = np.asarray(res_out_list[m]).reshape(NG, P, GB, NT)
        # o[g, p, j, t] = softmax score(batch m*B + g*GB + j, s = t*128 + p)
        outs.append(o.transpose(0, 2, 3, 1).reshape(B, SEQ))
    full = np.concatenate(outs, axis=0)[None]
    return np.ascontiguousarray(full.astype(np.float32))


def _run(queries, keys, **spmd_kwargs):
    from concourse import bass_utils

    nc = _get_program()
    in_maps = _make_in_maps(queries, keys)
    res = bass_utils.run_bass_kernel_spmd(
        nc, in_maps, core_ids=list(range(N_CORES)), **spmd_kwargs
    )
    out = _unpack_out([res.results[m]["out"] for m in range(N_CORES)])
    return out, res


def kernel(queries, keys):
    out, _ = _run(queries, keys)
    return out
